# revision 1
# baseline (speedup 1.0000x reference)
"""Trainium2 Bass kernel for nn_Attention_47124381171831.

Dense transformer block: 1x1-conv QKV projections (+BN eval), 8-head
attention over 1024 positions with a gathered relative-position bias,
ReLU, 1x1-conv output projection (+BN eval).

Sharding: pure data-parallel over batch. B=16 -> 2 batches per core on
8 cores, zero collectives. All weights / tables replicated.

Per-core design (2 batches):
  - all matmuls bf16 on PE (fp32 matmul is 4x slower per column)
  - BN affine + sqrt(softmax scale) folded into weights host-side
  - positional bias applied MULTIPLICATIVELY:
      softmax(qk*s + bias/s) = normalize(exp(qk*s) * E),
      E = exp(pos_tab/s)[pos_idx]  (host-expanded per head, streamed in)
  - dots computed transposed per head pair: S^T[j,i] = k^T q via
    row-tiled (K=32) matmuls into one [128, 2048] PSUM tile (4 banks)
  - exp on ACT directly PSUM->SBUF bf16, [128, 2048] per instruction
    (no max subtraction: dots are O(1) by construction)
  - DVE multiplies exp(dots) by streamed E tiles (bf16 4x mode)
  - attn@v transposed: O^T[dv,i] = v^T @ w2, with v produced already
    transposed by the V projection (lhsT = x). Head pair packed via
    column tiling (h -> out partitions 0:64, h' -> 64:128); softmax
    denominator Z via ones-matmuls into a parallel PSUM tile with the
    same partition layout, so the DVE normalize is lane-aligned.
  - normalization deferred across ReLU: relu(O/Z) = relu(O)/Z, applied
    pre-Wo; Wo + BN folded; f32 output.
"""

import sys

import numpy as np

sys.path.insert(0, "/opt/trn_rl_repo")

import ml_dtypes  # noqa: E402

BF16 = ml_dtypes.bfloat16

B, C, F, H, DK, DV = 16, 256, 32, 8, 32, 64
N = F * F  # 1024
EPS = 1e-5
SCALE = DK ** -0.5
NCORES = 8
BL = B // NCORES  # batches per core

_CACHE = {}


def _build_bass():
    import concourse.bacc as bacc
    import concourse.tile as tile
    from concourse import mybir

    f32 = mybir.dt.float32
    bf16 = mybir.dt.bfloat16
    AF = mybir.ActivationFunctionType

    nc = bacc.Bacc("TRN2", target_bir_lowering=False)

    x_d = nc.dram_tensor("x", [BL, 2, 128, N], bf16, kind="ExternalInput")
    wqk_d = nc.dram_tensor("wqk", [2, 128, 512], bf16, kind="ExternalInput")
    wv_d = nc.dram_tensor("wv", [2, 128, 512], bf16, kind="ExternalInput")
    wo_d = nc.dram_tensor("wo", [4, 128, 256], bf16, kind="ExternalInput")
    qkb_d = nc.dram_tensor("qkb", [1, 512], bf16, kind="ExternalInput")
    vb_d = nc.dram_tensor("vb", [1, 512], bf16, kind="ExternalInput")
    wob_d = nc.dram_tensor("wob", [1, 256], bf16, kind="ExternalInput")
    e_d = nc.dram_tensor("etab", [H, N, N], bf16, kind="ExternalInput")
    out_d = nc.dram_tensor("out", [BL, 2, 128, N], f32, kind="ExternalOutput")

    with tile.TileContext(nc) as tc:
        with (
            tc.tile_pool(name="const", bufs=1) as cpool,
            tc.tile_pool(name="acts", bufs=1) as apool,
            tc.tile_pool(name="etile", bufs=18) as e_pool,
            tc.tile_pool(name="wexp", bufs=3) as w_pool,
            tc.tile_pool(name="w2", bufs=3) as w2_pool,
            tc.tile_pool(name="post", bufs=2) as post_pool,
            tc.tile_pool(name="outs", bufs=2) as out_pool,
        ):
            # ---------- constants ----------
            wqk_sb = [cpool.tile([128, 512], bf16, name=f"wqk{i}", tag=f"wqk{i}")
                      for i in range(2)]
            wv_sb = [cpool.tile([128, 512], bf16, name=f"wv{i}", tag=f"wv{i}")
                     for i in range(2)]
            wo_sb = [cpool.tile([128, 256], bf16, name=f"wo{i}", tag=f"wo{i}")
                     for i in range(4)]
            qkb_sb = cpool.tile([1, 512], bf16, name="qkb", tag="qkb")
            vb_sb = cpool.tile([1, 512], bf16, name="vb", tag="vb")
            wob_sb = cpool.tile([1, 256], bf16, name="wob", tag="wob")
            ones_sb = cpool.tile([128, 64], bf16, name="ones", tag="ones")
            ones_row = cpool.tile([1, 1024], bf16, name="ones_row",
                                  tag="ones_row")
            for i in range(2):
                nc.sync.dma_start(wqk_sb[i][:], wqk_d[i])
                nc.sync.dma_start(wv_sb[i][:], wv_d[i])
            for i in range(4):
                nc.sync.dma_start(wo_sb[i][:], wo_d[i])
            nc.sync.dma_start(qkb_sb[:], qkb_d[:])
            nc.sync.dma_start(vb_sb[:], vb_d[:])
            nc.sync.dma_start(wob_sb[:], wob_d[:])
            nc.gpsimd.memset(ones_sb[:], 1.0)
            nc.gpsimd.memset(ones_row[:], 1.0)

            x_sb = [[apool.tile([128, N], bf16, name=f"x{b}{ck}", tag=f"x{b}{ck}")
                     for ck in range(2)] for b in range(BL)]
            q_sb = [[apool.tile([128, N], bf16, name=f"q{b}{ct}", tag=f"q{b}{ct}")
                     for ct in range(2)] for b in range(BL)]
            k_sb = [[apool.tile([128, N], bf16, name=f"k{b}{ct}", tag=f"k{b}{ct}")
                     for ct in range(2)] for b in range(BL)]
            # augmented v: even heads h=2hp at cols [hp*65, hp*65+65) as
            # [v|1]; odd heads h=2hp+1 at cols [260+hp*128, +128) as
            # [0(32)|1|0(31)|v] so Z lands on a 32-aligned PSUM row and
            # O_h1 on partitions 64:128.
            v_sb = [[apool.tile([128, 772], bf16, name=f"v{b}{jt}", tag=f"v{b}{jt}")
                     for jt in range(8)] for b in range(BL)]
            ro_sb = [[apool.tile([128, N], bf16, name=f"ro{b}{ctk}", tag=f"ro{b}{ctk}")
                      for ctk in range(4)] for b in range(BL)]
            for b in range(BL):
                for ck in range(2):
                    nc.sync.dma_start(x_sb[b][ck][:], x_d[b, ck])
            for b in range(BL):
                for jt in range(8):
                    t = v_sb[b][jt]
                    nc.gpsimd.memset(t[:, 260:772], 0.0)
                    nc.gpsimd.memset(t[:, 64:260:65], 1.0)
                    nc.gpsimd.memset(t[:, 292:772:128], 1.0)

            # ---------- phase 1: QKV projections ----------
            with tc.tile_pool(name="psQ", bufs=2, space="PSUM") as psQ:
                for b in range(BL):
                    for mt in range(4):  # 0,1 -> q ; 2,3 -> k
                        ps = psQ.tile([128, N], f32, name="qkps", tag="qkps")
                        for nh in range(2):
                            nc.tensor.matmul(
                                ps[:, nh * 512:(nh + 1) * 512],
                                qkb_sb[:, mt * 128:(mt + 1) * 128],
                                ones_row[:, nh * 512:(nh + 1) * 512],
                                start=True, stop=False,
                            )
                            for ck in range(2):
                                nc.tensor.matmul(
                                    ps[:, nh * 512:(nh + 1) * 512],
                                    wqk_sb[ck][:, mt * 128:(mt + 1) * 128],
                                    x_sb[b][ck][:, nh * 512:(nh + 1) * 512],
                                    start=False, stop=(ck == 1),
                                )
                        dst = q_sb[b][mt] if mt < 2 else k_sb[b][mt - 2]
                        nc.vector.tensor_copy(dst[:], ps[:])
                    for jt in range(8):
                        ps = psQ.tile([128, 512], f32, name="vps", tag="vps")
                        nc.tensor.matmul(
                            ps[:],
                            ones_row[:, jt * 128:(jt + 1) * 128],
                            vb_sb[:],
                            start=True, stop=False,
                        )
                        for ck in range(2):
                            nc.tensor.matmul(
                                ps[:],
                                x_sb[b][ck][:, jt * 128:(jt + 1) * 128],
                                wv_sb[ck][:],
                                start=False, stop=(ck == 1),
                            )
                        psv = ps[:].rearrange(
                            "p (hp par dv) -> p hp par dv", par=2, dv=64)
                        ev = v_sb[b][jt][:, 0:260].rearrange(
                            "p (hp c) -> p hp c", c=65)
                        od = v_sb[b][jt][:, 260:772].rearrange(
                            "p (hp c) -> p hp c", c=128)
                        nc.vector.tensor_copy(ev[:, :, 0:64], psv[:, :, 0, :])
                        nc.vector.tensor_copy(od[:, :, 64:128], psv[:, :, 1, :])

            # ---------- phase 2+3 share psA/psB ----------
            with (
                tc.tile_pool(name="psA", bufs=1, space="PSUM") as psA,
                tc.tile_pool(name="psB", bufs=1, space="PSUM") as psB,
            ):
                for hp in range(4):
                    h0 = 2 * hp
                    ct = hp // 2
                    rr = [(h0 % 4) * 32, (h0 % 4) * 32 + 32]
                    e_tiles = []
                    for jt in range(8):
                        et = e_pool.tile([128, 2, 1024], bf16, name="et",
                                         tag="et")
                        nc.sync.dma_start(
                            et[:],
                            e_d[h0:h0 + 2, jt * 128:(jt + 1) * 128, :]
                            .rearrange("h j i -> j h i"),
                        )
                        e_tiles.append(et[:].rearrange("j h i -> j (h i)"))

                    for b in range(BL):
                        av0 = psB.tile([128, N], f32, name="av0", tag="av0")
                        av1 = psB.tile([128, N], f32, name="av1", tag="av1")
                        for jt in range(8):
                            dots = psA.tile([128, 2048], f32, name="dots",
                                            tag="dots")
                            # same-ih pairs adjacent: the two PE row-groups
                            # (rr0, rr1) run concurrently
                            for ih in range(2):
                                for hi in range(2):
                                    nc.tensor.matmul(
                                        dots[:, hi * 1024 + ih * 512:
                                             hi * 1024 + (ih + 1) * 512],
                                        k_sb[b][ct][rr[hi]:rr[hi] + 32,
                                                    jt * 128:(jt + 1) * 128],
                                        q_sb[b][ct][rr[hi]:rr[hi] + 32,
                                                    ih * 512:(ih + 1) * 512],
                                        start=True, stop=True,
                                        tile_position=(rr[hi], 0),
                                    )
                            w = w_pool.tile([128, 2048], bf16, name="w", tag="w")
                            nc.scalar.activation(w[:], dots[:], AF.Exp)
                            w2 = w2_pool.tile([128, 2048], bf16, name="w2", tag="w2")
                            nc.vector.tensor_mul(w2[:], w[:], e_tiles[jt])
                            st, sp = (jt == 0), (jt == 7)

                            def wslice(hi, ih):
                                return w2[:, hi * 1024 + ih * 512:
                                          hi * 1024 + (ih + 1) * 512]

                            # augmented attn@v: h0 -> av0 rows 0:65
                            # (O 0:64, Z row 64); h1 -> av1 (Z row 32,
                            # O rows 64:128)
                            for ih in range(2):
                                osl = slice(ih * 512, (ih + 1) * 512)
                                nc.tensor.matmul(
                                    av0[0:65, osl],
                                    v_sb[b][jt][:, hp * 65:hp * 65 + 65],
                                    wslice(0, ih), start=st, stop=sp,
                                )
                                nc.tensor.matmul(
                                    av1[:, osl],
                                    v_sb[b][jt][:, 260 + hp * 128:
                                                260 + (hp + 1) * 128],
                                    wslice(1, ih), start=st, stop=sp,
                                )
                        # drain PSUM fast: recip Z rows + relu-copy O
                        # (relu commutes with the positive 1/Z scale), then
                        # normalize off the critical path in SBUF bf16.
                        rz = post_pool.tile([128, N], bf16, name="rz", tag="rz")
                        with nc.allow_low_precision(
                                "1/Z in bf16: 0.4%% rel on softmax scale"):
                            nc.vector.reciprocal(rz[64:65, :],
                                                 av0[64:65, :])
                            nc.vector.reciprocal(rz[32:33, :],
                                                 av1[32:33, :])
                        nc.vector.tensor_relu(ro_sb[b][hp][0:64, :],
                                              av0[0:64, :])
                        nc.vector.tensor_relu(ro_sb[b][hp][64:128, :],
                                              av1[64:128, :])
                        rzb = psB.tile([128, N], f32, name="rzb",
                                       tag="av0")
                        for ih in range(2):
                            osl = slice(ih * 512, (ih + 1) * 512)
                            nc.tensor.matmul(
                                rzb[0:64, osl], ones_sb[64:65, 0:64],
                                rz[64:65, osl], start=True, stop=True,
                                tile_position=(64, 0),
                            )
                            nc.tensor.matmul(
                                rzb[64:128, osl], ones_sb[32:33, 0:64],
                                rz[32:33, osl], start=True, stop=True,
                                tile_position=(32, 64),
                            )
                        nc.vector.tensor_mul(ro_sb[b][hp][:],
                                             ro_sb[b][hp][:], rzb[:])

            # ---------- phase 3: output projection ----------
            with tc.tile_pool(name="psW", bufs=2, space="PSUM") as psW:
                for b in range(BL):
                    for mt in range(2):
                        ps = psW.tile([128, N], f32, name="wops", tag="wops")
                        for ih in range(2):
                            nc.tensor.matmul(
                                ps[:, ih * 512:(ih + 1) * 512],
                                wob_sb[:, mt * 128:(mt + 1) * 128],
                                ones_row[:, ih * 512:(ih + 1) * 512],
                                start=True, stop=False,
                            )
                            for ctk in range(4):
                                nc.tensor.matmul(
                                    ps[:, ih * 512:(ih + 1) * 512],
                                    wo_sb[ctk][:, mt * 128:(mt + 1) * 128],
                                    ro_sb[b][ctk][:, ih * 512:(ih + 1) * 512],
                                    start=False, stop=(ctk == 3),
                                )
                        ot = out_pool.tile([128, N], f32, name="ot", tag="ot")
                        nc.vector.tensor_copy(ot[:], ps[:])
                        nc.sync.dma_start(out_d[b, mt], ot[:])

    nc.compile()
    return nc


def _prep_host(inputs):
    """Fold BN into weights, expand the bias table, build per-core maps."""
    x = np.asarray(inputs["x"], np.float32).reshape(B, C, N)
    Wq = np.asarray(inputs["Wq"], np.float32)
    Wk = np.asarray(inputs["Wk"], np.float32)
    Wv = np.asarray(inputs["Wv"], np.float32)
    Wo = np.asarray(inputs["Wo"], np.float32)
    bo = np.asarray(inputs["bo"], np.float32)
    pos_tab = np.asarray(inputs["pos_tab"], np.float32)
    pos_idx = np.asarray(inputs["pos_idx"])

    def fold(W, g, b_, m, v, gain=1.0):
        s = (np.asarray(g, np.float32)
             / np.sqrt(np.asarray(v, np.float32) + EPS))
        return W * (gain * s)[:, None], gain * (
            np.asarray(b_, np.float32) - np.asarray(m, np.float32) * s)

    ss = SCALE ** 0.5
    Wqf, bqf = fold(Wq, inputs["q_g"], inputs["q_b"], inputs["q_m"],
                    inputs["q_v"], ss)
    Wkf, bkf = fold(Wk, inputs["k_g"], inputs["k_b"], inputs["k_m"],
                    inputs["k_v"], ss)
    Wvf, bvf = fold(Wv, inputs["v_g"], inputs["v_b"], inputs["v_m"],
                    inputs["v_v"], 1.0)
    s_o = (np.asarray(inputs["o_g"], np.float32)
           / np.sqrt(np.asarray(inputs["o_v"], np.float32) + EPS))
    Wof = Wo * s_o[:, None]
    bof = (bo - np.asarray(inputs["o_m"], np.float32)) * s_o \
        + np.asarray(inputs["o_b"], np.float32)

    wqk = np.concatenate([Wqf.T, Wkf.T], axis=1)          # (256, 512)
    wv = np.ascontiguousarray(Wvf.T)                      # (256, 512)
    wo = np.ascontiguousarray(Wof.T)                      # (512, 256)

    etab = np.exp(pos_tab / SCALE).astype(np.float32)     # (N, H)
    e_full = etab[pos_idx.T, :]                           # (j, i, H)
    e_full = np.ascontiguousarray(e_full.transpose(2, 0, 1)).astype(BF16)

    common = dict(
        wqk=np.ascontiguousarray(wqk.reshape(2, 128, 512)).astype(BF16),
        wv=wv.reshape(2, 128, 512).astype(BF16),
        wo=wo.reshape(4, 128, 256).astype(BF16),
        qkb=np.concatenate([bqf, bkf]).reshape(1, 512).astype(BF16),
        vb=bvf.reshape(1, 512).astype(BF16),
        wob=bof.reshape(1, 256).astype(BF16),
        etab=e_full,
    )
    in_maps = []
    for c in range(NCORES):
        xs = x[c * BL:(c + 1) * BL].reshape(BL, 2, 128, N).astype(BF16)
        m = dict(common)
        m["x"] = np.ascontiguousarray(xs)
        in_maps.append(m)
    return in_maps


def _get_nc():
    if "nc" not in _CACHE:
        _CACHE["nc"] = _build_bass()
    return _CACHE["nc"]


def run(inputs, trace=False):
    from concourse.bass_utils import run_bass_kernel_spmd

    nc = _get_nc()
    in_maps = _prep_host(inputs)
    res = run_bass_kernel_spmd(
        nc, in_maps, core_ids=list(range(NCORES)), trace=trace)
    out = np.stack(
        [np.asarray(r["out"], np.float32).reshape(BL, C, N)
         for r in res.results], axis=0,
    ).reshape(B, C, F, F)
    return out, res


def kernel(**inputs):
    out, _ = run(inputs, trace=False)
    return out


if __name__ == "__main__":
    import reference

    ins = {k: np.asarray(v) for k, v in reference.setup_inputs().items()}
    exp = np.asarray(reference.reference(**ins))
    got = kernel(**ins)
    rel = np.linalg.norm(got - exp) / np.linalg.norm(exp)
    print("max abs err:", np.abs(got - exp).max(), "rel err:", rel)

